# revision 1
# baseline (speedup 1.0000x reference)
# Multi-head self-attention kernel for Trainium2, 8 NeuronCores.
# Sharding: data-parallel over batch (b=8 -> one batch per core).
# All inputs pre-transposed on host; zero on-device transposes.
#
# Per core (batch b), with hsT = hs[b].T [E, L], wqT/wkT/wvT = w.T [E, E]:
#   qT[e_out, l] = sum_e wqT[e, e_out] * hsT[e, l]      (lhsT=wqT chunk, rhs=hsT)  fp32r
#   kT likewise; v[l, e_out] = sum_e hsT[e, l] * wvT[e, e_out] (lhsT=hsT, rhs=wvT) fp32r
#   scoresT[lk, lq] = sum_d kT[d, lk] * qT[d, lq]  per head (K=64, two heads
#     packed per 128-partition tile -> concurrent row-tiled matmuls)
#   expT = exp(scoresT / 8)  (no max-subtraction needed: |scores| small)  -> fp16
#   pv[lq, 0:65] = sum_lk expT[lk, lq] * [v_h | 1][lk, 0:65]   (ones col -> row sums)
#   out_h[lq, d] = pv[lq, d] / pv[lq, 64] + b_v[h*64+d]
# Output DRAM [H, L, D] per core == reference's out.reshape(L, H*D) bytes.

import os
import numpy as np

B, L, E = 8, 1024, 1024
H, D = 16, 64
NC = 8          # cores
P = 128         # partitions
CH = E // P     # 8 contraction chunks
MT = E // P     # 8 output tiles (e_out or l)
LT = L // P     # 8 l-tiles
HPT = P // D    # 2 heads per 128-partition tile
VW = D + 2      # 66: v cols per head (64 + ones col + pad for 4B alignment)

TRACE = False
DEBUG = False
_cached = {}


def _build():
    import concourse.bacc as bacc
    import concourse.mybir as mybir
    import concourse.tile as tile

    F32 = mybir.dt.float32
    F32R = mybir.dt.float32r
    F16 = mybir.dt.float16
    Exp = mybir.ActivationFunctionType.Exp

    nc = bacc.Bacc("TRN2", target_bir_lowering=False, debug=False)
    hsT = nc.dram_tensor("hsT", [E, L], F32R, kind="ExternalInput").ap()
    wqT = nc.dram_tensor("wqT", [E, E], F32R, kind="ExternalInput").ap()
    wkT = nc.dram_tensor("wkT", [E, E], F32R, kind="ExternalInput").ap()
    wvT = nc.dram_tensor("wvT", [E, E], F32R, kind="ExternalInput").ap()
    bq = nc.dram_tensor("bq", [E], F32, kind="ExternalInput").ap()
    bk = nc.dram_tensor("bk", [E], F32, kind="ExternalInput").ap()
    bv = nc.dram_tensor("bv", [E], F32, kind="ExternalInput").ap()
    out = nc.dram_tensor("out", [H, L, D], F32, kind="ExternalOutput").ap()
    dbg = {}
    if DEBUG:
        dbg["q"] = nc.dram_tensor("dbg_q", [P, MT, L], F16, kind="ExternalOutput").ap()
        dbg["k"] = nc.dram_tensor("dbg_k", [P, MT, L], F16, kind="ExternalOutput").ap()
        dbg["v"] = nc.dram_tensor("dbg_v", [P, LT, H * VW], F16, kind="ExternalOutput").ap()
        dbg["e"] = nc.dram_tensor("dbg_e", [LT, HPT, P, L], F16, kind="ExternalOutput").ap()
        dbg["pv"] = nc.dram_tensor("dbg_pv", [HPT, P, 8 * P], F32, kind="ExternalOutput").ap()

    import concourse.bass as bass

    with tile.TileContext(nc) as tc:
        with tc.tile_pool(name="big", bufs=1) as big, \
             tc.tile_pool(name="wpool", bufs=3) as wpool, \
             tc.tile_pool(name="epool", bufs=20) as epool, \
             tc.tile_pool(name="spool", bufs=3) as spool, \
             tc.tile_pool(name="psum", bufs=2, space="PSUM") as pp:

            # ---- constants / biases ----
            bq_sb = big.tile([P, MT], F32)
            bk_sb = big.tile([P, MT], F32)
            nc.sync.dma_start(out=bq_sb, in_=bq.rearrange("(m p) -> p m", p=P))
            nc.sync.dma_start(out=bk_sb, in_=bk.rearrange("(m p) -> p m", p=P))
            bv_bc = big.tile([P, E], F32)
            nc.sync.dma_start(
                out=bv_bc,
                in_=bass.AP(tensor=bv.tensor, offset=0, ap=[[0, P], [1, E]]),
            )

            # ---- resident SBUF tensors ----
            hsT_sb = big.tile([P, CH, L], F32R)     # [p, c, l]
            for c in range(CH):
                nc.sync.dma_start(out=hsT_sb[:, c, :], in_=hsT[c * P:(c + 1) * P, :])
            wv_sb = big.tile([P, CH, E], F32R)      # [p, c, e_out]
            for c in range(CH):
                nc.sync.dma_start(out=wv_sb[:, c, :], in_=wvT[c * P:(c + 1) * P, :])

            qT_sb = big.tile([P, MT, L], F16)       # [p(e_out in tile), m, lq]
            kT_sb = big.tile([P, MT, L], F16)
            v_sb = big.tile([P, LT, H * VW], F16)   # [p(l in tile), m, h*65+c]

            # ones columns of v (written once; PV's ones-column row sums)
            v4 = v_sb.rearrange("p m (h c) -> p m h c", h=H)
            nc.vector.memset(v4[:, :, :, D:VW], 1.0)

            # ---- V projection: v[l, e_out] (lhsT = hsT chunk, rhs = wvT) ----
            for m in range(LT):
                ps = pp.tile([P, 1024], F32, tag="mm", name=f"psv{m}")
                for c in range(CH):
                    for n in range(2):
                        nc.tensor.matmul(
                            ps[:, n * 512:(n + 1) * 512],
                            hsT_sb[:, c, m * P:(m + 1) * P],
                            wv_sb[:, c, n * 512:(n + 1) * 512],
                            start=(c == 0), stop=(c == CH - 1),
                        )
                nc.vector.tensor_copy(
                    v4[:, m, :, 0:D],
                    ps.rearrange("p (h c) -> p h c", h=H),
                )

            # ---- Q/K projections: qT[e_out, l] (lhsT = wT chunk, rhs = hsT) ----
            for m in range(MT):
                for (wT, dst, bias) in ((wqT, qT_sb, bq_sb), (wkT, kT_sb, bk_sb)):
                    wt = wpool.tile([P, CH, P], F32R, tag="w", name=f"w{m}")
                    nc.sync.dma_start(
                        out=wt,
                        in_=wT[:, m * P:(m + 1) * P].rearrange(
                            "(c p) n -> p c n", p=P),
                    )
                    ps = pp.tile([P, 1024], F32, tag="mm", name=f"psqk{m}")
                    for c in range(CH):
                        for n in range(2):
                            nc.tensor.matmul(
                                ps[:, n * 512:(n + 1) * 512],
                                wt[:, c, :],
                                hsT_sb[:, c, n * 512:(n + 1) * 512],
                                start=(c == 0), stop=(c == CH - 1),
                            )
                    nc.vector.tensor_scalar_add(dst[:, m, :], ps, bias[:, m:m + 1])

            if DEBUG:
                nc.sync.dma_start(out=dbg["q"], in_=qT_sb)
                nc.sync.dma_start(out=dbg["k"], in_=kT_sb)
                nc.sync.dma_start(out=dbg["v"], in_=v_sb)

            # ---- attention, one head-pair (= one qT/kT tile) at a time ----
            for p_i in range(MT):
                heads = (2 * p_i, 2 * p_i + 1)
                exps = [[], []]   # per half: expT tiles per lk chunk
                for lk in range(LT):
                    scs = []
                    for half in range(HPT):
                        lo, hi = half * D, (half + 1) * D
                        sc = pp.tile([P, 1024], F32, tag="mm",
                                     name=f"sc{p_i}_{lk}_{half}")
                        for n in range(2):
                            nc.tensor.matmul(
                                sc[:, n * 512:(n + 1) * 512],
                                kT_sb[lo:hi, p_i, lk * P:(lk + 1) * P],
                                qT_sb[lo:hi, p_i, n * 512:(n + 1) * 512],
                                start=True, stop=True,
                            )
                        scs.append(sc)
                    for half in range(HPT):
                        e_t = epool.tile([P, 1024], F16, tag="e",
                                         name=f"e{p_i}_{lk}_{half}")
                        nc.scalar.activation(e_t, scs[half], Exp, scale=0.125)
                        exps[half].append(e_t)
                        if DEBUG and p_i == 0 and not os.environ.get("DBG_NO_E"):
                            est = spool.tile([P, 1024], F16, tag="est",
                                             name=f"est{lk}_{half}")
                            nc.vector.tensor_copy(est, e_t)
                            nc.sync.dma_start(out=dbg["e"][lk, half], in_=est)

                pvs = [pp.tile([P, 8, P], F32, tag="pv", name=f"pv{p_i}_{h}")
                       for h in range(HPT)]
                for half in range(HPT):
                    h = heads[half]
                    for t in range(LT):
                        for c in range(LT):
                            nc.tensor.matmul(
                                pvs[half][:, t, 0:D + 1],
                                exps[half][c][:, t * P:(t + 1) * P],
                                v_sb[:, c, h * VW:h * VW + D + 1],
                                start=(c == 0), stop=(c == LT - 1),
                            )

                if DEBUG == 2 and p_i == 0:
                    for half in range(HPT):
                        pvc = spool.tile([P, 8 * P], F32, tag="pvc",
                                         name=f"pvc{half}")
                        nc.vector.tensor_copy(
                            pvc, pvs[half].rearrange("p a b -> p (a b)"))
                        nc.sync.dma_start(out=dbg["pv"][half], in_=pvc)

                # normalize + bias + store
                for half in range(HPT):
                    h = heads[half]
                    pv = pvs[half]
                    rs = spool.tile([P, LT], F32, tag="rs", name=f"rs{p_i}_{half}")
                    nc.vector.reciprocal(rs, pv[:, :, D:D + 1].squeeze(2))
                    st = spool.tile([P, LT, D], F32, tag="st",
                                    name=f"st{p_i}_{half}")
                    for t in range(LT):
                        nc.vector.tensor_scalar_mul(
                            st[:, t, :], pv[:, t, 0:D], rs[:, t:t + 1])
                    nc.vector.tensor_add(
                        st,
                        st,
                        bv_bc[:, None, h * D:(h + 1) * D].broadcast_to([P, LT, D]),
                    )
                    nc.sync.dma_start(
                        out=out[h].rearrange("(t p) d -> p t d", p=P),
                        in_=st,
                    )

    nc.compile()
    return nc


def _get_nc():
    if "nc" not in _cached:
        _cached["nc"] = _build()
    return _cached["nc"]


def kernel(hidden_states, w_q, b_q, w_k, b_k, w_v, b_v):
    from concourse import bass_utils

    hs = np.asarray(hidden_states, dtype=np.float32)
    w_q = np.asarray(w_q, dtype=np.float32)
    w_k = np.asarray(w_k, dtype=np.float32)
    w_v = np.asarray(w_v, dtype=np.float32)
    b_q = np.asarray(b_q, dtype=np.float32)
    b_k = np.asarray(b_k, dtype=np.float32)
    b_v = np.asarray(b_v, dtype=np.float32)

    nc = _get_nc()
    hsT = np.ascontiguousarray(hs.transpose(0, 2, 1))
    wqT = np.ascontiguousarray(w_q.T)
    wkT = np.ascontiguousarray(w_k.T)
    wvT = np.ascontiguousarray(w_v.T)
    in_maps = [
        {"hsT": hsT[i], "wqT": wqT, "wkT": wkT, "wvT": wvT,
         "bq": b_q, "bk": b_k, "bv": b_v}
        for i in range(NC)
    ]
    res = bass_utils.run_bass_kernel_spmd(
        nc, in_maps, core_ids=list(range(NC)), trace=TRACE)
    kernel.last_exec_time_ns = res.exec_time_ns
    kernel.last_results = res.results
    return np.stack([res.results[i]["out"].reshape(L, H * D) for i in range(NC)])


kernel.last_exec_time_ns = None



# revision 3
# speedup vs baseline: 1.2718x; 1.2718x over previous
# Multi-head self-attention kernel for Trainium2, 8 NeuronCores.
# Sharding: data-parallel over batch (b=8 -> one batch per core).
#
# v2: software-pipelined single pass, fp16 inputs.
#  - All inputs cast to fp16 on host (halves DMA, enables FWL weight loads,
#    matmuls run at 1 col/cycle either way).
#  - Emission order interleaves QK projection / scores / exp / PV per
#    head-pair so the PE never idles long enough for HAM to re-throttle and
#    the ACT engine's exp stream (the second-longest engine chain) hides
#    under PE work.
#  - Per core (batch b), with hsT = hs[b].T [E, L] fp16:
#      qT[e_out, l], kT[e_out, l] = wT.T @ hsT + bias    (fp16, per m-tile)
#      v[l, e_out] = hsT.T @ wvT                         (fp16, + ones cols)
#      scoresT[lk, lq] = kT_h.T @ qT_h  per head, 2 heads row-packed
#      expT = exp(scoresT / 8) -> fp16 SBUF
#      pv[lq, 0:65] = expT_chunk.T @ [v_h | 1]  (ones col -> softmax denom)
#      out_h[lq, d] = pv[lq, d] / pv[lq, 64] + b_v[h*64+d]   (fp16 out)
# Output DRAM [H, L, D] per core == reference's out.reshape(L, H*D) bytes.

import numpy as np

B, L, E = 8, 1024, 1024
H, D = 16, 64
NC = 8          # cores
P = 128         # partitions
CH = E // P     # 8 contraction chunks
MT = E // P     # 8 output tiles (e_out) == head pairs
LT = L // P     # 8 l-tiles
HPT = P // D    # 2 heads per 128-partition tile
VW = D + 2      # 66: v cols per head (64 + ones col + pad)

TRACE = False
_cached = {}


def _build():
    import concourse.bacc as bacc
    import concourse.mybir as mybir
    import concourse.tile as tile
    import concourse.bass as bass

    F32 = mybir.dt.float32
    F16 = mybir.dt.float16
    Exp = mybir.ActivationFunctionType.Exp
    Mult = mybir.AluOpType.mult
    Add = mybir.AluOpType.add

    nc = bacc.Bacc("TRN2", target_bir_lowering=False, debug=False)
    hsT = nc.dram_tensor("hsT", [E, L], F16, kind="ExternalInput").ap()
    wqT = nc.dram_tensor("wqT", [E, E], F16, kind="ExternalInput").ap()
    wkT = nc.dram_tensor("wkT", [E, E], F16, kind="ExternalInput").ap()
    wvT = nc.dram_tensor("wvT", [E, E], F16, kind="ExternalInput").ap()
    bq = nc.dram_tensor("bq", [E], F32, kind="ExternalInput").ap()
    bk = nc.dram_tensor("bk", [E], F32, kind="ExternalInput").ap()
    bv = nc.dram_tensor("bv", [E], F32, kind="ExternalInput").ap()
    out = nc.dram_tensor("out", [H, L, D], F16, kind="ExternalOutput").ap()

    with tile.TileContext(nc) as tc:
        with tc.tile_pool(name="big", bufs=1) as big, \
             tc.tile_pool(name="wpool", bufs=4) as wpool, \
             tc.tile_pool(name="epool", bufs=6) as epool, \
             tc.tile_pool(name="spool", bufs=3) as spool, \
             tc.tile_pool(name="pjp", bufs=2, space="PSUM") as pjp, \
             tc.tile_pool(name="scp", bufs=1, space="PSUM") as scp, \
             tc.tile_pool(name="pvp", bufs=2, space="PSUM") as pvp:

            # ---- input DMAs (sync ring: hsT + first weight slices first;
            #      scalar/ACT ring: wv + remaining weight slices) ----
            hsT_sb = big.tile([P, CH, L], F16)
            nc.sync.dma_start(
                out=hsT_sb, in_=hsT.rearrange("(c p) l -> p c l", p=P))

            wts = {}

            def load_w(m, which, eng):
                wT = wqT if which == 0 else wkT
                t = wpool.tile([P, CH, P], F16, tag="w", name=f"w{which}_{m}")
                eng.dma_start(
                    out=t,
                    in_=wT[:, m * P:(m + 1) * P].rearrange(
                        "(c p) n -> p c n", p=P))
                wts[(m, which)] = t

            load_w(0, 0, nc.sync)
            load_w(0, 1, nc.sync)
            load_w(1, 0, nc.sync)
            load_w(1, 1, nc.sync)

            wv_sb = big.tile([P, CH, E], F16)
            nc.scalar.dma_start(
                out=wv_sb, in_=wvT.rearrange("(c p) l -> p c l", p=P))

            bq_sb = big.tile([P, MT], F32)
            bk_sb = big.tile([P, MT], F32)
            nc.sync.dma_start(out=bq_sb, in_=bq.rearrange("(m p) -> p m", p=P))
            nc.sync.dma_start(out=bk_sb, in_=bk.rearrange("(m p) -> p m", p=P))
            bv_bc = big.tile([P, E], F32)
            nc.sync.dma_start(
                out=bv_bc,
                in_=bass.AP(tensor=bv.tensor, offset=0, ap=[[0, P], [1, E]]))

            # ---- resident SBUF tensors ----
            qT_sb = big.tile([P, MT, L], F16)       # [p(e_out in tile), m, lq]
            kT_sb = big.tile([P, MT, L], F16)
            v_sb = big.tile([P, LT, H * VW], F16)   # [p(l in tile), m, h*66+c]
            v4 = v_sb.rearrange("p m (h c) -> p m h c", h=H)
            nc.vector.memset(v4[:, :, :, D:VW], 1.0)

            # ---- emit helpers (emission order == per-engine exec order) ----
            def emit_qk(m):
                # prefetch weights two head-pairs ahead on the ACT ring
                if m + 2 < MT:
                    load_w(m + 2, 0, nc.scalar)
                    load_w(m + 2, 1, nc.scalar)
                for which, (dst, bias) in enumerate(
                        ((qT_sb, bq_sb), (kT_sb, bk_sb))):
                    wt = wts.pop((m, which))
                    for n in range(2):
                        ps = pjp.tile([P, 512], F32, tag="pj",
                                      name=f"psqk{m}_{which}_{n}")
                        for c in range(CH):
                            nc.tensor.matmul(
                                ps, wt[:, c, :],
                                hsT_sb[:, c, n * 512:(n + 1) * 512],
                                start=(c == 0), stop=(c == CH - 1))
                        nc.vector.tensor_scalar_add(
                            dst[:, m, n * 512:(n + 1) * 512], ps,
                            bias[:, m:m + 1])

            def emit_v(m):
                for n in range(2):
                    ps = pjp.tile([P, 512], F32, tag="pj", name=f"psv{m}_{n}")
                    for c in range(CH):
                        nc.tensor.matmul(
                            ps, hsT_sb[:, c, m * P:(m + 1) * P],
                            wv_sb[:, c, n * 512:(n + 1) * 512],
                            start=(c == 0), stop=(c == CH - 1))
                    nc.vector.tensor_copy(
                        v4[:, m, n * 8:(n + 1) * 8, 0:D],
                        ps.rearrange("p (h c) -> p h c", h=8))

            def emit_sc_chunk(p_i, c, exps):
                # scores chunk c (lk tile) for both head halves, row-packed:
                # half 0 uses PE rows 0:64, half 1 rows 64:128 (concurrent)
                for half in range(HPT):
                    lo, hi = half * D, (half + 1) * D
                    sc = scp.tile([P, L], F32, tag=f"sch{half}",
                                  name=f"sc{p_i}_{c}_{half}")
                    for n in range(2):
                        nc.tensor.matmul(
                            sc[:, n * 512:(n + 1) * 512],
                            kT_sb[lo:hi, p_i, c * P:(c + 1) * P],
                            qT_sb[lo:hi, p_i, n * 512:(n + 1) * 512],
                            start=True, stop=True)
                    nc.scalar.activation(
                        exps[half][:, c, :], sc, Exp, scale=0.125)

            def alloc_exps(p_i):
                return [epool.tile([P, CH, L], F16, tag="e",
                                   name=f"e{p_i}_{h}") for h in range(HPT)]

            def emit_pv_t(p_i, t, exps, st):
                # pv[lq, 0:65] for both halves at lq-tile t; 65th col = denom
                pv = pvp.tile([P, HPT, 68], F32, tag="pv", name=f"pv{p_i}_{t}")
                for half in range(HPT):
                    h = 2 * p_i + half
                    for c in range(CH):
                        nc.tensor.matmul(
                            pv[:, half, 0:D + 1],
                            exps[half][:, c, t * P:(t + 1) * P],
                            v_sb[:, c, h * VW:h * VW + D + 1],
                            start=(c == 0), stop=(c == CH - 1))
                for half in range(HPT):
                    h = 2 * p_i + half
                    rs = spool.tile([P, 1], F32, tag="rs",
                                    name=f"rs{p_i}_{t}_{half}")
                    nc.vector.reciprocal(rs, pv[:, half, D:D + 1])
                    nc.vector.scalar_tensor_tensor(
                        st[:, half, t, :], pv[:, half, 0:D], rs,
                        bv_bc[:, h * D:(h + 1) * D], Mult, Add)

            def alloc_st(p_i):
                return spool.tile([P, HPT, LT, D], F16, tag="st",
                                  name=f"st{p_i}")

            def emit_out(p_i, st):
                for half in range(HPT):
                    h = 2 * p_i + half
                    nc.sync.dma_start(
                        out=out[h].rearrange("(t p) d -> p t d", p=P),
                        in_=st[:, half])

            # ---- pipelined emission ----
            # PE stream: QK0 QK1 | SC0+V0..3 | SC1+V4..7 |
            #            QK2 SC2+PV0 | QK3 SC3+PV1 | ... | QK7 SC7+PV5 |
            #            PV6 PV7
            emit_qk(0)
            emit_qk(1)

            exps = {0: alloc_exps(0)}
            sts = {}
            for c in range(CH):
                emit_sc_chunk(0, c, exps[0])
                if c % 2 == 1:
                    emit_v(c // 2)
            exps[1] = alloc_exps(1)
            for c in range(CH):
                emit_sc_chunk(1, c, exps[1])
                if c % 2 == 1:
                    emit_v(4 + c // 2)
            for p_i in range(2, MT):
                emit_qk(p_i)
                exps[p_i] = alloc_exps(p_i)
                sts[p_i - 2] = alloc_st(p_i - 2)
                for c in range(CH):
                    emit_sc_chunk(p_i, c, exps[p_i])
                    emit_pv_t(p_i - 2, c, exps[p_i - 2], sts[p_i - 2])
                emit_out(p_i - 2, sts[p_i - 2])
                del exps[p_i - 2]
            for p_i in (MT - 2, MT - 1):
                sts[p_i] = alloc_st(p_i)
                for t in range(LT):
                    emit_pv_t(p_i, t, exps[p_i], sts[p_i])
                emit_out(p_i, sts[p_i])

    nc.compile()
    return nc


def _get_nc():
    if "nc" not in _cached:
        _cached["nc"] = _build()
    return _cached["nc"]


def kernel(hidden_states, w_q, b_q, w_k, b_k, w_v, b_v):
    from concourse import bass_utils

    hs = np.asarray(hidden_states, dtype=np.float32)
    b_q = np.asarray(b_q, dtype=np.float32)
    b_k = np.asarray(b_k, dtype=np.float32)
    b_v = np.asarray(b_v, dtype=np.float32)

    nc = _get_nc()
    hsT = np.ascontiguousarray(
        hs.transpose(0, 2, 1)).astype(np.float16)
    wqT = np.ascontiguousarray(np.asarray(w_q, np.float32).T).astype(np.float16)
    wkT = np.ascontiguousarray(np.asarray(w_k, np.float32).T).astype(np.float16)
    wvT = np.ascontiguousarray(np.asarray(w_v, np.float32).T).astype(np.float16)
    in_maps = [
        {"hsT": hsT[i], "wqT": wqT, "wkT": wkT, "wvT": wvT,
         "bq": b_q, "bk": b_k, "bv": b_v}
        for i in range(NC)
    ]
    res = bass_utils.run_bass_kernel_spmd(
        nc, in_maps, core_ids=list(range(NC)), trace=TRACE)
    kernel.last_exec_time_ns = res.exec_time_ns
    kernel.last_results = res.results
    return np.stack(
        [res.results[i]["out"].reshape(L, H * D).astype(np.float32)
         for i in range(NC)])


kernel.last_exec_time_ns = None
